# revision 3
# baseline (speedup 1.0000x reference)
"""Trainium2 Bass kernel for a 2-layer dense-adjacency GAT (nn_GAT_17824114278677).

Sharding: nodes (rows of the attention matrix) are sharded across the 8
NeuronCores, 512 rows per core; weights and node features are replicated.
Two SPMD launches (one per GAT layer) with a host-side gather of the layer-1
output in between.

Per-core dataflow: attention tiles are computed TRANSPOSED, [j=128
partitions, r=512 rows], so the aggregation att @ Wh maps directly onto the
PE (contraction over j on partitions) with zero on-chip transposes.

Key identity (cheaper than the old exp-max-exp formulation): softmax is
invariant to a per-row scale, so divide the whole row by exp(s_r) and fold
exp(d_j) into the stationary Wh.  The remaining per-element factor is

    p[j,r] = exp(leaky_relu(t) - t) * m[j,r]     (t = s_r + d_j)
           = max(1, exp(-0.8 t)) * m[j,r]

computable two ways, assigned per key-chunk to balance ScalarE vs VectorE:
  path V (VectorE only): q = (zb[r] * w[j]) max 1   (one fused tensor_scalar,
      zb = exp(-0.8 s) broadcast, w = exp(-0.8 d) per-partition);
      p = q * mask01                               (grouped tensor_tensor)
  path S (ScalarE + VectorE): q = Exp(zln[r] + bias_j)  (one activation,
      zln = -0.8 s broadcast, bias = -0.8 d per-partition);
      p = (q max 1) * mask01                       (grouped scalar_tensor_tensor)

softmax denominators ride along as the exp(d) column in the stationary
operand; division + ELU happen on the host on the tiny per-head
[HID+1, 512] outputs.

Wh = x @ W (0.4%% of the FLOPs) plus the per-node attention vectors
f_src/f_dst are computed on the host in fp32 and shipped pre-rounded to
bf16; all on-device attention/aggregation math runs in bf16 with fp32 PSUM
accumulation.
"""

import os
import sys
import time
from contextlib import ExitStack

for _p in ("/opt/trn_rl_repo", "/root/.axon_site/_ro/trn_rl_repo"):
    if os.path.isdir(_p) and _p not in sys.path:
        sys.path.append(_p)

import numpy as np
import ml_dtypes

import bass_rust
import concourse.bass as bass
import concourse.tile as tile
from concourse import mybir
from concourse.bass_utils import run_bass_kernel_spmd

BF16 = ml_dtypes.bfloat16
F32 = mybir.dt.float32
BF = mybir.dt.bfloat16

N = 4096          # nodes
NCORES = 8
R = N // NCORES   # rows (queries) per core
CJ = N // 128     # 32 key chunks
FIN = 512         # input feature dim of both layers
NV = 13           # key chunks routed to path V (VectorE-only); rest path S
NV2 = 13          # same for layer 2
GRP = 4           # chunk-group size for fused VectorE ops

CORE_IDS = list(range(NCORES))

LAST_PERF = {}


# ---------------------------------------------------------------------------
# walrus workaround: it rejects instructions carrying >1 sync-wait command
# ("Too many sync wait commands").  Move excess waits onto preceding
# same-engine NoOps -- semantically identical (same-engine waits are totally
# ordered before the instruction).
def _split_excess_waits(nc, max_waits: int = 1) -> int:
    n_split = 0
    for fn in nc.m.functions:
        for bb in fn.blocks:
            insts = bb.instructions
            new_insts = []
            changed = False
            for ins in insts:
                si = ins.sync_info
                waits = list(si.on_wait) if si is not None else []
                if len(waits) > max_waits:
                    extra, keep = waits[:-max_waits], waits[-max_waits:]
                    for k in range(0, len(extra), max_waits):
                        chunk = extra[k : k + max_waits]
                        nop = bass_rust.InstNoOp(
                            name=f"{ins.name}-wsplit{k}", ins=[], outs=[]
                        )
                        nop.engine = ins.engine
                        nop.sync_info = mybir.SyncInfo(on_wait=chunk, on_update=[])
                        new_insts.append(nop)
                        n_split += 1
                    si.on_wait = keep
                    changed = True
                new_insts.append(ins)
            if changed:
                bb.instructions = new_insts
    return n_split


# ---------------------------------------------------------------------------
def _build_layer(H: int, HID: int, nv: int = NV):
    """One GAT layer, per-core program.

    Inputs (per core):
      whxin  [128, CJ, H, WPH] bf16  Wh*exp(d) per head + exp(d) column
      maskM  [128, CJ, R]   bf16  multiplicative 0/1 adjacency, transposed
      zbB    [128, H, R]    bf16  exp(-0.8 f_src) of this core's rows (bcast)
      zlnB   [128, H, R]    bf16  -0.8 f_src (bcast)
      wcol   [128, H*CJ]    f32   [p, h*CJ+c] = exp(-0.8 f_dst[h, 128c+p])
      fdst2  [128, H*CJ]    f32   -0.8 f_dst
    Output:
      agg    [H, HID+1, R]  f32   rows 0..HID-1: unnormalized att @ Wh
                                  (transposed); row HID: softmax denominator
    """
    WPH = HID + 2  # per-head stride in Whx: HID cols + exp(d) col + pad

    nc = bass.Bass("TRN2", debug=False, num_devices=NCORES)
    whxin = nc.dram_tensor("whxin", [128, CJ, H, WPH], BF, kind="ExternalInput")
    maskM = nc.dram_tensor("maskM", [128, CJ, R], BF, kind="ExternalInput")
    zbB = nc.dram_tensor("zbB", [128, H, R], BF, kind="ExternalInput")
    zlnB = nc.dram_tensor("zlnB", [128, H, R], BF, kind="ExternalInput")
    wcol = nc.dram_tensor("wcol", [128, H * CJ], F32, kind="ExternalInput")
    fdst2 = nc.dram_tensor("fdst2", [128, H * CJ], F32, kind="ExternalInput")
    agg = nc.dram_tensor("agg", [H, HID + 1, R], F32, kind="ExternalOutput")

    EXP = mybir.ActivationFunctionType.Exp
    MAX = mybir.AluOpType.max
    MUL = mybir.AluOpType.mult

    with tile.TileContext(nc) as tc, ExitStack() as ctx:
        cpool = ctx.enter_context(tc.tile_pool(name="const", bufs=1))
        wpool = ctx.enter_context(tc.tile_pool(name="whx", bufs=1))
        tpool = ctx.enter_context(tc.tile_pool(name="work", bufs=3))
        opool = ctx.enter_context(tc.tile_pool(name="out", bufs=2))
        paq = ctx.enter_context(tc.tile_pool(name="psa", bufs=3, space="PSUM"))

        # ---- resident constants -------------------------------------------
        zb_t = cpool.tile([128, H, R], BF, tag="zb")
        nc.sync.dma_start(zb_t[:], zbB[:])
        zln_t = cpool.tile([128, H, R], BF, tag="zln")
        nc.sync.dma_start(zln_t[:], zlnB[:])
        w_t = cpool.tile([128, H * CJ], F32, tag="wcol")
        nc.sync.dma_start(w_t[:], wcol[:])
        fdst2_t = cpool.tile([128, H * CJ], F32, tag="fdst2")
        nc.sync.dma_start(fdst2_t[:], fdst2[:])
        mask_t = cpool.tile([128, CJ, R], BF, tag="mask")

        # mask + whx interleaved so the first chunks of both arrive early
        NMQ = 8
        whx = [None] * CJ
        for mq in range(NMQ):
            cs = slice(mq * (CJ // NMQ), (mq + 1) * (CJ // NMQ))
            nc.sync.dma_start(mask_t[:, cs, :], maskM[:, cs, :])
            for c in range(cs.start, cs.stop):
                wx = wpool.tile([128, H, WPH], BF, tag=f"whx{c}", name=f"whx{c}")
                nc.sync.dma_start(wx[:], whxin[:, c])
                whx[c] = wx

        # per-head chunk-group schedule: V-path and S-path groups interleaved
        # so both engines always have independent work in flight
        vgrps, sgrps = [], []
        for lo, hi in ((0, nv), (nv, CJ)):
            c = lo
            while c < hi:
                g = min(GRP, hi - c)
                (vgrps if lo == 0 else sgrps).append((c, g, lo == 0))
                c += g
        groups = []
        for i in range(max(len(vgrps), len(sgrps))):
            if i < len(sgrps):
                groups.append(sgrps[i])
            if i < len(vgrps):
                groups.append(vgrps[i])

        # ---- attention + aggregation --------------------------------------
        for h in range(H):
            pa = paq.tile([HID + 1, R], F32, tag="psa")
            for gi, (c0, G, is_v) in enumerate(groups):
                p3p = tpool.tile([128, GRP, R], BF, tag="p3")
                qp = tpool.tile([128, GRP, R], BF, tag="q")
                if is_v:
                    # path V: q = (zb * w_j) max 1 ; p = q * m01
                    for k in range(G):
                        o_ix = h * CJ + c0 + k
                        nc.vector.tensor_scalar(
                            qp[:, k, :], zb_t[:, h, :],
                            w_t[:, o_ix : o_ix + 1], 1.0, op0=MUL, op1=MAX,
                        )
                    nc.vector.tensor_tensor(
                        p3p[:, 0:G, :], qp[:, 0:G, :],
                        mask_t[:, c0 : c0 + G, :], op=MUL,
                    )
                else:
                    # path S: q = exp(zln + (-0.8 d_j)) ; p = (q max 1) * m01
                    for k in range(G):
                        o_ix = h * CJ + c0 + k
                        nc.scalar.activation(
                            qp[:, k, :], zln_t[:, h, :], EXP,
                            bias=fdst2_t[:, o_ix : o_ix + 1], scale=1.0,
                        )
                    nc.vector.scalar_tensor_tensor(
                        p3p[:, 0:G, :], qp[:, 0:G, :], 1.0,
                        mask_t[:, c0 : c0 + G, :], op0=MAX, op1=MUL,
                    )
                for k in range(G):
                    c = c0 + k
                    nc.tensor.matmul(
                        pa[:], whx[c][:, h, 0 : HID + 1], p3p[:, k, :],
                        start=(gi == 0 and k == 0),
                        stop=(gi == len(groups) - 1 and k == G - 1),
                    )
            o = opool.tile([HID + 1, R], F32, tag="aggo")
            nc.vector.tensor_copy(o[:], pa[:])
            nc.sync.dma_start(agg[h], o[:])

    return nc


_PROGS = {}


def _get_prog(H, HID, nv=NV):
    """Build (and cache) the layer program with the walrus wait-split fix
    applied.  The fix is HW-only: CoreSim's event loop rejects the injected
    NoOps, so sim users should call _build_layer directly."""
    key = (H, HID, nv)
    if key not in _PROGS:
        nc = _build_layer(H, HID, nv)
        _split_excess_waits(nc)
        _PROGS[key] = nc
    return _PROGS[key]


def _elu(v):
    return np.where(v > 0, v, np.expm1(np.minimum(v, 0.0))).astype(np.float32)


def _host_inputs(f_src, f_dst, adj, Wh, H):
    """Shared per-layer host prep.  f_src/f_dst [N, H] f32, adj [N, N] i32,
    Wh [N, H*HID] f32 (pre-activation per-head features)."""
    HID = Wh.shape[1] // H
    WPH = HID + 2
    fdst_arr = np.ascontiguousarray(
        f_dst.T.reshape(H, CJ, 128).transpose(2, 0, 1).reshape(128, H * CJ)
    ).astype(np.float32)
    fdst2_arr = (-0.8 * fdst_arr).astype(np.float32)
    w_arr = np.exp(fdst2_arr).astype(np.float32)       # exp(-0.8 f_dst)

    # exp(f_dst) folded into the stationary operand; ones-col becomes exp(d)
    ev = np.exp(f_dst).astype(np.float32)  # [N, H]
    whx = np.zeros((128, CJ, H, WPH), np.float32)
    whx[:, :, :, :HID] = (
        (Wh.reshape(N, H, HID) * ev[:, :, None])
        .reshape(CJ, 128, H, HID).transpose(1, 0, 2, 3)
    )
    whx[:, :, :, HID] = ev.reshape(CJ, 128, H).transpose(1, 0, 2)

    shared = {
        "fdst2": fdst2_arr,
        "wcol": w_arr,
        "whxin": whx.astype(BF16),
    }
    per_core = []
    for i in range(NCORES):
        rows = slice(R * i, R * (i + 1))
        adjT = adj[rows, :].T.astype(np.float32)  # [N, R] 0/1
        fs = np.ascontiguousarray(f_src[rows, :].T)  # [H, R]
        zln = -0.8 * fs
        d = dict(shared)
        d["maskM"] = np.ascontiguousarray(
            adjT.reshape(CJ, 128, R).transpose(1, 0, 2)
        ).astype(BF16)
        d["zbB"] = np.broadcast_to(
            np.exp(zln)[None, :, :], (128, H, R)
        ).astype(BF16)
        d["zlnB"] = np.broadcast_to(zln[None, :, :], (128, H, R)).astype(BF16)
        per_core.append(d)
    return per_core


def _run_layer(nc, in_maps, H, HID, tag):
    t0 = time.time()
    res = run_bass_kernel_spmd(nc, in_maps, core_ids=CORE_IDS)
    LAST_PERF[f"{tag}_wall_s"] = time.time() - t0
    LAST_PERF[f"{tag}_exec_ns"] = res.exec_time_ns

    hT = np.empty((H * HID, N), np.float32)
    for i in range(NCORES):
        a = res.results[i]["agg"]  # [H, HID+1, R]
        denom = a[:, HID : HID + 1, :]
        hT[:, R * i : R * (i + 1)] = (a[:, :HID, :] / denom).reshape(H * HID, R)
    return hT


def kernel(x, adj, W1, a1, W2, a2):
    x = np.asarray(x, np.float32)
    adj = np.asarray(adj, np.int32)
    W1 = np.asarray(W1, np.float32)
    a1 = np.asarray(a1, np.float32)
    W2 = np.asarray(W2, np.float32)
    a2 = np.asarray(a2, np.float32)

    H1, HID1, OUT = W1.shape[0], W1.shape[2], W2.shape[1]

    progA = _get_prog(H1, HID1)
    progB = _get_prog(1, OUT, NV2)

    # ---- layer 1 ----------------------------------------------------------
    W1c = np.ascontiguousarray(W1.transpose(1, 0, 2).reshape(FIN, H1 * HID1))
    wsrc1 = np.einsum("hfk,hk->fh", W1, a1[:, :HID1, 0]).astype(np.float32)
    wdst1 = np.einsum("hfk,hk->fh", W1, a1[:, HID1:, 0]).astype(np.float32)
    f_src1 = x @ wsrc1  # [N, H]
    f_dst1 = x @ wdst1
    Wh1 = x @ W1c  # [N, H1*HID1]

    in_maps = _host_inputs(f_src1, f_dst1, adj, Wh1, H1)
    hT = _run_layer(progA, in_maps, H1, HID1, "layer1")
    hcatT = _elu(hT)  # [512, N] == h_cat.T (concat=True applies elu)

    # ---- layer 2 ----------------------------------------------------------
    hcat = np.ascontiguousarray(hcatT.T)  # [N, 512]
    wsrc2 = (W2 @ a2[:OUT, 0]).astype(np.float32)[:, None]
    wdst2 = (W2 @ a2[OUT:, 0]).astype(np.float32)[:, None]
    f_src2 = hcat @ wsrc2  # [N, 1]
    f_dst2 = hcat @ wdst2
    Wh2 = hcat @ W2  # [N, OUT]
    in_maps2 = _host_inputs(f_src2, f_dst2, adj, Wh2, 1)
    outT = _run_layer(progB, in_maps2, 1, OUT, "layer2")
    # layer 2: concat=False -> no inner elu; final output = elu(out)
    return np.ascontiguousarray(_elu(outT).T)
